# revision 1
# baseline (speedup 1.0000x reference)
"""Trainium2 Bass kernel: stereo cost-volume builder.

cv[b, d, h, w] = mean_c( feat_l[b, c, h, w] * feat_r[b, c, h, w - d] ),
zero where w < d.  B=8, C=128, H=128, W=256, D=48.

Strategy (data-parallel over batch, one sample per NeuronCore):
  For each h, the cost volume row is a 48-wide band of the per-row Gram
  matrix G = L_h^T R_h (contraction over C=128 on the TensorE partition
  dim).  Two band-restricted fp32 matmuls per h fill one PSUM bank:
    psum[:,   0:128] = L[:,h,   0:128]^T @ R[:,h,  0:128]   (N=128)
    psum[:, 128:304] = L[:,h, 128:256]^T @ R[:,h, 80:256]   (N=176)
  VectorE drains the 304-col band to SBUF; a GpSimd local_scatter with a
  precomputed per-partition index table (fp32 treated as uint16 pairs)
  extracts the 48 diagonals into Y[p, (chunk, d)] (invalid w<d slots are
  zeroed by the scatter's dst[:]=0).  A TensorE transpose turns Y into
  [d-major, w] layout in PSUM, ScalarE drains it (scaling by 1/C for the
  mean), and one strided DMA per (4h, chunk) writes cv[d, h, w] to HBM.
"""

import numpy as np

import concourse.bass as bass
import concourse.mybir as mybir
import concourse.tile as tile
from concourse import bacc, library_config
from concourse.bass_utils import run_bass_kernel_spmd
from concourse.masks import make_identity

F32 = mybir.dt.float32
U16 = mybir.dt.uint16
I16 = mybir.dt.int16

B, C, H, W, D = 8, 128, 128, 256, 48
NMM0, NMM1 = 128, 176
BANDW = NMM0 + NMM1  # 304
N_CORES = 8


def _make_idx_table():
    """int16 [128, 608] scatter table mapping band uint16 pairs -> Y pairs.

    band col q in [0,128):   P0[p,q] = cv_sum[p-q,   h, p]     (0 <= p-q < 48)
    band col q in [128,304): m=q-128, P1[p,m] = cv_sum[48+p-m, h, 128+p]
    Y[p, d] = cv chunk0, Y[p, 48+d] = cv chunk1.
    """
    idx = np.full((128, 2 * BANDW), -1, np.int16)
    for p in range(128):
        for n in range(NMM0):
            d = p - n
            if 0 <= d < D:
                idx[p, 2 * n] = 2 * d
                idx[p, 2 * n + 1] = 2 * d + 1
        for m in range(NMM1):
            n = NMM0 + m
            d = 48 + p - m
            if 0 <= d < D:
                idx[p, 2 * n] = 2 * (D + d)
                idx[p, 2 * n + 1] = 2 * (D + d) + 1
    return idx


def _build(nc, tc, l_ap, r_ap, idx_ap, out_ap, HB=16, HM=4):
    HWs = H * W
    with (
        tc.tile_pool(name="lio", bufs=3) as lpool,
        tc.tile_pool(name="rio", bufs=3) as rpool,
        tc.tile_pool(name="bandp", bufs=4) as bandpool,
        tc.tile_pool(name="yp", bufs=4) as ypool,
        tc.tile_pool(name="ytsb", bufs=2) as ytsbpool,
        tc.tile_pool(name="misc", bufs=1) as misc,
        tc.tile_pool(name="gp", bufs=5, space="PSUM") as gpool,
        tc.tile_pool(name="ytp", bufs=2, space="PSUM") as ytpool,
    ):
        ident = misc.tile([128, 128], F32)
        make_identity(nc, ident[:])
        itab = misc.tile([128, 2 * BANDW], I16)
        nc.sync.dma_start(itab[:], idx_ap)

        for hb in range(H // HB):
            lblk = lpool.tile([128, HB * W], F32, tag="l")
            rblk = rpool.tile([128, HB * W], F32, tag="r")
            nc.sync.dma_start(lblk[:], l_ap[:, hb * HB * W:(hb + 1) * HB * W])
            nc.sync.dma_start(rblk[:], r_ap[:, hb * HB * W:(hb + 1) * HB * W])
            for hm in range(HB // HM):
                ytps = ytpool.tile([96, HM * 128], F32, tag="yt")
                for t in range(HM):
                    o = (hm * HM + t) * W
                    g = gpool.tile([128, 512], F32, tag="g")
                    nc.tensor.matmul(g[:, 0:NMM0], lblk[:, o:o + 128],
                                     rblk[:, o:o + NMM0], start=True, stop=True)
                    nc.tensor.matmul(g[:, NMM0:BANDW], lblk[:, o + 128:o + 256],
                                     rblk[:, o + 80:o + 256], start=True, stop=True)
                    band = bandpool.tile([128, BANDW], F32, tag="band")
                    nc.vector.tensor_copy(band[:], g[:, 0:BANDW])
                    y = ypool.tile([128, 96], F32, tag="y")
                    nc.gpsimd.local_scatter(y[:].bitcast(U16), band[:].bitcast(U16),
                                            itab[:], channels=128,
                                            num_elems=192, num_idxs=2 * BANDW)
                    nc.tensor.transpose(ytps[:, t * 128:(t + 1) * 128], y[:], ident[:])
                yt_sb = ytsbpool.tile([96, HM * 128], F32, tag="ytsb")
                nc.scalar.mul(yt_sb[:], ytps[:], 1.0 / C)
                h0 = hb * HB + hm * HM
                for c in range(2):
                    src = bass.AP(yt_sb.tensor, 48 * c * (HM * 128),
                                  [[HM * 128, 48], [128, HM], [1, 128]])
                    dst = bass.AP(out_ap.tensor, h0 * W + 128 * c,
                                  [[HWs, 48], [W, HM], [1, 128]])
                    nc.sync.dma_start(dst, src)


_CACHE = {}


def _get_nc():
    if "nc" not in _CACHE:
        nc = bacc.Bacc("TRN2", target_bir_lowering=False, debug=False,
                       num_devices=N_CORES)
        l_ap = nc.dram_tensor("l", [C, H * W], F32, kind="ExternalInput").ap()
        r_ap = nc.dram_tensor("r", [C, H * W], F32, kind="ExternalInput").ap()
        idx_ap = nc.dram_tensor("idx", [128, 2 * BANDW], I16,
                                kind="ExternalInput").ap()
        out_ap = nc.dram_tensor("out", [D, H * W], F32, kind="ExternalOutput").ap()
        with tile.TileContext(nc, trace_sim=False) as tc:
            nc.gpsimd.load_library(library_config.local_scatter)
            _build(nc, tc, l_ap, r_ap, idx_ap, out_ap)
        nc.compile()
        _CACHE["nc"] = nc
        _CACHE["idx"] = _make_idx_table()
    return _CACHE["nc"], _CACHE["idx"]


def kernel(feat_l: np.ndarray, feat_r: np.ndarray, **run_kwargs) -> np.ndarray:
    feat_l = np.ascontiguousarray(np.asarray(feat_l), dtype=np.float32)
    feat_r = np.ascontiguousarray(np.asarray(feat_r), dtype=np.float32)
    assert feat_l.shape == (B, C, H, W), feat_l.shape
    nc, idx = _get_nc()
    in_maps = [
        {"l": feat_l[b].reshape(C, H * W),
         "r": feat_r[b].reshape(C, H * W),
         "idx": idx}
        for b in range(B)
    ]
    res = run_bass_kernel_spmd(nc, in_maps, core_ids=list(range(N_CORES)),
                               **run_kwargs)
    out = np.stack([res.results[b]["out"].reshape(D, H, W) for b in range(B)])
    if run_kwargs.get("trace"):
        kernel.last_results = res
    return out

